# revision 15
# baseline (speedup 1.0000x reference)
"""MoE top-1 routing layer on 8 Trainium2 NeuronCores.

The reference layer has NO nonlinearity between the expert matmul and the
output projection, so per routed token
    out[t] = x[t] @ W[e] @ OW + (b[e] @ OW + ob),   e = argmax(x[t] @ GW + gb)
The weight-only products are fused on the host (exactly like the baseline
already fused the bias path bias2[e] = b[e] @ OW + ob):
    FW[e] = W[e] @ OW            # [D, O] per expert, input-independent
so each device core runs a single bf16 matmul per token: out = x_tok @ FW.

Sharding: each core owns one "home" expert (up to CAP_A=970 of its tokens)
plus one 74-token overflow slot holding spillover tokens of some other
expert (second FW buffer). With the balanced capacities every core does
64*(970+74) = 66.8k PE cycles (~28us at 2.4GHz) instead of 64*1118 for
plain expert-parallel. Tokens that still don't fit (never, for this
routing) fall back to a host matmul.

Device schedule (from trace analysis of v1):
- each dma_start costs ~650ns serialized on its issuing engine's queue, so
  inputs are batched into ~10 transfers and split across the two HWDGE
  queues (sync + scalar) so issue latency never gates the stream;
- a diagonal sweep over (o-tile, token-group) visits keeps per-visit new
  DMA bytes under the ~360GB/s per-core HBM rate;
- ~12 junk matmuls at t=0 ramp the PE DVFS p-state (0.65->2.4GHz takes
  ~3us of continuous busy; gaps reset it) while the first operands land;
- PSUM->SBUF copies alternate vector/gpsimd so no engine saturates and
  the scalar queue stays free for DMA issue (also avoids its 1.3us
  ACT_TABLE_LOAD preamble).
"""

import numpy as np
from contextlib import ExitStack

B, S, D, E, H, O = 4, 2048, 1024, 8, 2048, 1024
T = B * S
P = 128
KO_D = D // P      # 8  contraction tiles
KO_O = O // P      # 8  output tiles
CAP_A = 970        # home-expert token slot per core
CAP_B = 74         # overflow-expert token slot per core
GA = [128, 128, 256, 458]   # A-slot token groups (PSUM free dim <= 512)
assert sum(GA) == CAP_A
N_JUNK = 14        # p-state warm-up matmuls ([P,256] each)
N_JUNK64 = 4       # fine-grained warm-up tail ([P,64] each)

# Visit order: groups 0..3 = A groups, 4 = B group. Small a/b/c groups run
# first in a diagonal sweep (per-visit new-data rate under the DMA stream
# rate), the big d groups + tiny B groups follow once the bulk has landed.
# Groups a,b,c of one ot accumulate into disjoint column ranges of a single
# PSUM bank, so each ot needs just one 512-col PSUM->SBUF copy.
VISITS = [
    (0, 0), (1, 0), (0, 1), (2, 0), (1, 1), (3, 0), (2, 1), (0, 2),
    (4, 0), (3, 1), (1, 2), (5, 0), (4, 1), (2, 2), (6, 0), (5, 1),
    (3, 2), (7, 0), (6, 1), (4, 2), (7, 1), (5, 2), (6, 2), (7, 2),
    (0, 3), (1, 3), (2, 3), (0, 4), (3, 3), (1, 4), (4, 3), (2, 4),
    (5, 3), (3, 4), (6, 3), (4, 4), (7, 3), (5, 4), (6, 4), (7, 4),
]
assert sorted(VISITS) == sorted((ot, g) for ot in range(8) for g in range(5))


def _legalize_waits(nc):
    """This container's walrus accepts 1 sem wait per instruction (2 for
    EventSemaphore); Tile's tail drain can carry more. Split the excess
    onto preceding same-engine NoOps."""
    from concourse import mybir

    uid = 0
    for f in nc.m.functions:
        for b in f.blocks:
            insts = b.instructions
            out = []
            changed = False
            for ins in insts:
                si = ins.sync_info
                waits = list(si.on_wait) if si is not None else []
                limit = 2 if str(ins.opcode) == "EventSemaphore" else 1
                if len(waits) > limit:
                    extra, keep = waits[:-limit], waits[-limit:]
                    for w in extra:
                        uid += 1
                        out.append(
                            mybir.InstNoOp(
                                name=f"waitsplit-{uid}",
                                engine=ins.engine,
                                sync_info=mybir.SyncInfo(on_wait=[w], on_update=[]),
                                bass_nofuse=True,
                            )
                        )
                    si.on_wait = keep
                    changed = True
                out.append(ins)
            if changed:
                insts.clear()
                insts.extend(out)


def _patch_tail_barrier(tile_mod):
    """Tile's kernel tail is drain -> barrier -> sem-reset -> barrier.
    The second all-engine barrier only orders the sem-reset against program
    end, which the per-engine stream end already guarantees; drop it."""
    if getattr(tile_mod.TileContext, "_moe_tail_patched", False):
        return
    from concourse.vector_clock import ScopedClock

    def _drain_and_barrier(self, tick_clock, wait_clock):
        drain_inst = self.nc.sync.drain()
        wait_clock.add_sem_waits(
            drain_inst.ins, ScopedClock({None: tick_clock.global_clock})
        )
        self.nc.all_engine_barrier()
        popped = self.nc._tile_sem_poison_stack.pop()
        assert popped is self._sem_poison
        self.nc.clear_and_free_semaphores(list(self.sems.allocated().values()))

    tile_mod.TileContext._drain_and_barrier = _drain_and_barrier
    tile_mod.TileContext._moe_tail_patched = True


def _patch_walrus_policy():
    """Compile with walrus --policy=2 (heuristics post-scheduler): measured
    faster than the default --policy=0 on this kernel family."""
    import concourse.bass_utils as bu

    if getattr(bu, "_moe_policy_patched", False):
        return
    orig = bu.run_command

    def _rc(argv, **kw):
        if argv and "walrus_driver" in str(argv[0]):
            argv = ["--policy=2" if a == "--policy=0" else a for a in argv]
        return orig(argv, **kw)

    bu.run_command = _rc
    bu._moe_policy_patched = True


def _emit(nc, tile, mm_dt, f32):
    fwa = nc.dram_tensor("fwa", [P, KO_O, KO_D, P], mm_dt, kind="ExternalInput")
    fwb = nc.dram_tensor("fwb", [P, KO_O, KO_D, P], mm_dt, kind="ExternalInput")
    xa = nc.dram_tensor("xa", [P, KO_D, CAP_A], mm_dt, kind="ExternalInput")
    xb = nc.dram_tensor("xb", [P, KO_D, CAP_B], mm_dt, kind="ExternalInput")
    outa = nc.dram_tensor("outa", [P, KO_O, CAP_A], mm_dt, kind="ExternalOutput")
    outb = nc.dram_tensor("outb", [P, KO_O, CAP_B], mm_dt, kind="ExternalOutput")

    with tile.TileContext(nc) as tc:
        with ExitStack() as ctx:
            fw_pool = ctx.enter_context(tc.tile_pool(name="fw", bufs=1))
            x_pool = ctx.enter_context(tc.tile_pool(name="x", bufs=1))
            o_pool = ctx.enter_context(tc.tile_pool(name="o", bufs=1))
            jk_pool = ctx.enter_context(tc.tile_pool(name="jk", bufs=1))
            psa_pool = ctx.enter_context(
                tc.tile_pool(name="psa", bufs=5, space="PSUM")
            )
            ps_pool = ctx.enter_context(
                tc.tile_pool(name="ps", bufs=2, space="PSUM")
            )
            jp_pool = ctx.enter_context(
                tc.tile_pool(name="jp", bufs=1, space="PSUM")
            )

            fwa_sb = fw_pool.tile([P, KO_O, KO_D, P], mm_dt, name="fwa")
            fwb_sb = fw_pool.tile([P, KO_O, KO_D, P], mm_dt, name="fwb")
            xa_sb = x_pool.tile([P, KO_D, CAP_A], mm_dt, name="xa")
            xb_sb = x_pool.tile([P, KO_D, CAP_B], mm_dt, name="xb")
            outa_sb = o_pool.tile([P, KO_O, CAP_A], mm_dt, name="outa")
            outb_sb = o_pool.tile([P, KO_O, CAP_B], mm_dt, name="outb")
            jst = jk_pool.tile([P, P], mm_dt, name="jst")
            jmv = jk_pool.tile([P, 256], mm_dt, name="jmv")
            jps = jp_pool.tile([P, 512], f32, name="jps")

            # ---- junk warm-up: ramp the PE p-state while DMA streams in
            nc.vector.memset(jst[:], 0)
            nc.vector.memset(jmv[:], 0)
            for _ in range(N_JUNK):
                nc.tensor.matmul(jps[:, :256], jst[:], jmv[:], start=True,
                                 stop=True)
            for _ in range(N_JUNK64):
                nc.tensor.matmul(jps[:, :64], jst[:], jmv[:, :64], start=True,
                                 stop=True)

            # ---- input DMAs: ONE queue (sync), strict demand order. The
            # HWDGE ring paces issues to transfer completions and queues
            # share the ~360GB/s HBM port, so a single in-order stream beats
            # splitting (a second queue halves the critical stream's BW).
            # Output DMAs go on the scalar queue so they never block inputs.
            c01 = GA[0]
            c12 = GA[0] + GA[1]
            c23 = GA[0] + GA[1] + GA[2]
            # first visit gated on just 0.375MB: half of fwa ot0 + 128 cols;
            # stream strictly in first-need order
            nc.sync.dma_start(fwa_sb[:, 0, 0:4], fwa[:, 0, 0:4])
            nc.sync.dma_start(xa_sb[:, :, :c01], xa[:, :, :c01])
            nc.sync.dma_start(fwa_sb[:, 0, 4:8], fwa[:, 0, 4:8])
            nc.sync.dma_start(fwa_sb[:, 1], fwa[:, 1])
            nc.sync.dma_start(xa_sb[:, :, c01:c12], xa[:, :, c01:c12])
            nc.sync.dma_start(fwa_sb[:, 2], fwa[:, 2])
            nc.sync.dma_start(fwa_sb[:, 3], fwa[:, 3])
            nc.sync.dma_start(xa_sb[:, :, c12:c23], xa[:, :, c12:c23])
            for h in range(4, KO_O):
                nc.sync.dma_start(fwa_sb[:, h], fwa[:, h])
            nc.sync.dma_start(xa_sb[:, :, c23:], xa[:, :, c23:])
            nc.sync.dma_start(xb_sb[:], xb[:])
            nc.sync.dma_start(fwb_sb[:, 0:4], fwb[:, 0:4])
            nc.sync.dma_start(fwb_sb[:, 4:8], fwb[:, 4:8])

            groups = [  # (x tile, col0, width, fw tile, out tile)
                (xa_sb, 0, GA[0], fwa_sb, outa_sb),
                (xa_sb, c01, GA[1], fwa_sb, outa_sb),
                (xa_sb, c12, GA[2], fwa_sb, outa_sb),
                (xa_sb, c23, GA[3], fwa_sb, outa_sb),
                (xb_sb, 0, CAP_B, fwb_sb, outb_sb),
            ]
            psA = {}             # ot -> shared PSUM bank for groups a,b,c
            ddone = [False] * KO_O
            for ot, g in VISITS:
                x_sb, c0, gw, fw_sb, out_sb = groups[g]
                if g < 3:
                    if g == 0:
                        psA[ot] = psa_pool.tile([P, 512], f32, name="psa")
                    ps = psA[ot][:, c0 : c0 + gw]
                else:
                    ps = ps_pool.tile([P, 512], f32, name="ps")[:, :gw]
                for k in range(KO_D):
                    nc.tensor.matmul(
                        ps,
                        fw_sb[:, ot, k],
                        x_sb[:, k, c0 : c0 + gw] if g < 4 else x_sb[:, k],
                        start=(k == 0),
                        stop=(k == KO_D - 1),
                    )
                # copies on DVE only: gpsimd/Pool can't read PSUM on trn2, and
                # scalar ACTIVATE would pull in a 1.3us ACT_TABLE_LOAD that
                # delays the scalar queue's DMA issues
                if g == 2:  # a,b,c of this ot all accumulated -> one copy
                    nc.vector.tensor_copy(outa_sb[:, ot, :c23], psA.pop(ot)[:])
                elif g == 3:
                    nc.vector.tensor_copy(outa_sb[:, ot, c23:], ps)
                    ddone[ot] = True
                    pair = ot & ~1
                    if ddone[pair] and ddone[pair + 1]:
                        nc.scalar.dma_start(
                            outa[:, pair : pair + 2], outa_sb[:, pair : pair + 2]
                        )
                elif g == 4:
                    nc.vector.tensor_copy(outb_sb[:, ot, :], ps)
                    # outb rides the idle sync queue, in two halves so the
                    # final transfer after the last visit is small
                    if ot == 3:
                        nc.sync.dma_start(outb[:, 0:4], outb_sb[:, 0:4])
                    elif ot == 7:
                        nc.sync.dma_start(outb[:, 4:8], outb_sb[:, 4:8])
    return nc


def _build_nc():
    import concourse.bass as bass
    import concourse.tile as tile
    from concourse import mybir

    _patch_tail_barrier(tile)
    _patch_walrus_policy()
    nc = bass.Bass()
    _emit(nc, tile, mybir.dt.bfloat16, mybir.dt.float32)
    _legalize_waits(nc)
    return nc


_NC_CACHE = {}


def kernel(x, gate_w, gate_b, expert_w, expert_b, out_w, out_b):
    import os

    plats = os.environ.get("JAX_PLATFORMS")
    if plats and "axon" not in plats:
        os.environ["JAX_PLATFORMS"] = plats + ",axon"

    import ml_dtypes
    from concourse.bass_utils import run_bass_kernel_spmd

    bf = ml_dtypes.bfloat16
    x = np.asarray(x, dtype=np.float32)
    gate_w = np.asarray(gate_w, dtype=np.float32)
    gate_b = np.asarray(gate_b, dtype=np.float32)
    expert_w = np.asarray(expert_w, dtype=np.float32)
    expert_b = np.asarray(expert_b, dtype=np.float32)
    out_w = np.asarray(out_w, dtype=np.float32)
    out_b = np.asarray(out_b, dtype=np.float32)

    xt = x.reshape(T, D)
    # Gate on host in fp64: argmax matches the fp32 reference exactly
    # (min top-2 logit gap is ~1e-5, fp64 error ~1e-12).
    logits = xt.astype(np.float64) @ gate_w.astype(np.float64) + gate_b.astype(
        np.float64
    )
    idx = np.argmax(logits, axis=1)

    # Weight-only fusion (input-independent): FW[e] = W[e] @ OW, and the
    # bias path bias2[e] = b[e] @ OW + ob, both applied per routed token.
    FW = (expert_w.reshape(E * D, H) @ out_w).reshape(E, D, O)
    bias2 = (
        expert_b.astype(np.float64) @ out_w.astype(np.float64)
        + out_b.astype(np.float64)
    ).astype(np.float32)

    # fw packed [P, KO_O, KO_D, P]: fw[p, ot, kd, j] = FW[kd*128+p, ot*128+j]
    fw_pk = [
        np.ascontiguousarray(
            FW[e].astype(bf).reshape(KO_D, P, KO_O, P).transpose(1, 2, 0, 3)
        )
        for e in range(E)
    ]

    # ---- token -> core assignment: home expert e on core e (CAP_A tokens),
    # spillover chopped into <=CAP_B chunks placed in other cores' B slots.
    tok_of_expert = [np.nonzero(idx == e)[0] for e in range(E)]
    home = [t[:CAP_A] for t in tok_of_expert]
    ovf = [(e, tok_of_expert[e][CAP_A:]) for e in range(E)]
    chunks = []
    for e, t in sorted(ovf, key=lambda p: -len(p[1])):
        for i in range(0, len(t), CAP_B):
            chunks.append((e, t[i : i + CAP_B]))
    chunks = [c for c in chunks if len(c[1])]
    host_fb = chunks[E:]          # shouldn't happen for this routing
    chunks = chunks[:E]
    while len(chunks) < E:
        chunks.append((len(chunks), np.empty(0, dtype=np.int64)))

    def pack_x(toks, cap):
        cols = np.zeros((cap, D), dtype=bf)
        if len(toks):
            cols[: len(toks)] = xt[toks].astype(bf)
        # [cap, D] -> [P, KO_D, cap]
        return np.ascontiguousarray(
            cols.T.reshape(KO_D, P, cap).transpose(1, 0, 2)
        )

    in_maps = []
    for e in range(E):
        be, bt = chunks[e]
        in_maps.append(
            {
                "fwa": fw_pk[e],
                "fwb": fw_pk[be],
                "xa": pack_x(home[e], CAP_A),
                "xb": pack_x(bt, CAP_B),
            }
        )

    if "nc" not in _NC_CACHE:
        _NC_CACHE["nc"] = _build_nc()
    nc = _NC_CACHE["nc"]

    res = run_bass_kernel_spmd(nc, in_maps, list(range(E)))

    out = np.empty((T, O), dtype=np.float32)
    for e in range(E):
        # dev out [P, KO_O, cap] -> [cap, O] with col = ot*128+p
        oa = (
            np.asarray(res.results[e]["outa"])
            .transpose(2, 1, 0)
            .reshape(CAP_A, O)
            .astype(np.float32)
        )
        out[home[e]] = oa[: len(home[e])] + bias2[e]
        be, bt = chunks[e]
        if len(bt):
            ob_ = (
                np.asarray(res.results[e]["outb"])
                .transpose(2, 1, 0)
                .reshape(CAP_B, O)
                .astype(np.float32)
            )
            out[bt] = ob_[: len(bt)] + bias2[be]
    for e, toks in host_fb:
        out[toks] = xt[toks] @ FW[e] + bias2[e]
    return out.reshape(B, S, O)


# revision 20
# speedup vs baseline: 1.0841x; 1.0841x over previous
"""MoE top-1 routing layer on 8 Trainium2 NeuronCores.

The reference layer has NO nonlinearity between the expert matmul and the
output projection, so per routed token
    out[t] = x[t] @ W[e] @ OW + (b[e] @ OW + ob),   e = argmax(x[t] @ GW + gb)
The weight-only products are fused on the host (exactly like the baseline
already fused the bias path bias2[e] = b[e] @ OW + ob):
    FW[e] = W[e] @ OW            # [D, O] per expert, input-independent
so each device core runs a single bf16 matmul per token: out = x_tok @ FW.

Sharding: each core owns one "home" expert (up to CAP_A=970 of its tokens)
plus one 74-token overflow slot holding spillover tokens of some other
expert (second FW buffer). With the balanced capacities every core does
64*(970+74) = 66.8k PE cycles (~28us at 2.4GHz) instead of 64*1118 for
plain expert-parallel. Tokens that still don't fit (never, for this
routing) fall back to a host matmul.

Device schedule (from trace analysis of v1):
- each dma_start costs ~650ns serialized on its issuing engine's queue, so
  inputs are batched into ~10 transfers and split across the two HWDGE
  queues (sync + scalar) so issue latency never gates the stream;
- a diagonal sweep over (o-tile, token-group) visits keeps per-visit new
  DMA bytes under the ~360GB/s per-core HBM rate;
- ~12 junk matmuls at t=0 ramp the PE DVFS p-state (0.65->2.4GHz takes
  ~3us of continuous busy; gaps reset it) while the first operands land;
- PSUM->SBUF copies alternate vector/gpsimd so no engine saturates and
  the scalar queue stays free for DMA issue (also avoids its 1.3us
  ACT_TABLE_LOAD preamble).
"""

import numpy as np
from contextlib import ExitStack

B, S, D, E, H, O = 4, 2048, 1024, 8, 2048, 1024
T = B * S
P = 128
KO_D = D // P      # 8  contraction tiles
KO_O = O // P      # 8  output tiles
CAP_A = 970        # home-expert token slot per core
CAP_B = 74         # overflow-expert token slot per core
GA = [128, 128, 256, 458]   # A-slot token groups (PSUM free dim <= 512)
assert sum(GA) == CAP_A
N_JUNK = 14        # p-state warm-up matmuls ([P,256] each)
N_JUNK64 = 4       # fine-grained warm-up tail ([P,64] each)

# Visit order: groups 0..3 = A groups, 4 = B group. Small a/b/c groups run
# first in a diagonal sweep (per-visit new-data rate under the DMA stream
# rate), the big d groups + tiny B groups follow once the bulk has landed.
# Groups a,b,c of one ot accumulate into disjoint column ranges of a single
# PSUM bank, so each ot needs just one 512-col PSUM->SBUF copy.
VISITS = [
    (0, 0), (1, 0), (0, 1), (2, 0), (1, 1), (3, 0), (2, 1), (0, 2),
    (4, 0), (3, 1), (1, 2), (5, 0), (4, 1), (2, 2), (6, 0), (5, 1),
    (3, 2), (7, 0), (6, 1), (4, 2), (7, 1), (5, 2), (6, 2), (7, 2),
    (0, 3), (1, 3), (2, 3), (0, 4), (3, 3), (1, 4), (4, 3), (2, 4),
    (5, 3), (3, 4), (6, 3), (4, 4), (7, 3), (5, 4), (6, 4), (7, 4),
]
assert sorted(VISITS) == sorted((ot, g) for ot in range(8) for g in range(5))


def _legalize_waits(nc):
    """This container's walrus accepts 1 sem wait per instruction (2 for
    EventSemaphore); Tile's tail drain can carry more. Split the excess
    onto preceding same-engine NoOps."""
    from concourse import mybir

    uid = 0
    for f in nc.m.functions:
        for b in f.blocks:
            insts = b.instructions
            out = []
            changed = False
            for ins in insts:
                si = ins.sync_info
                waits = list(si.on_wait) if si is not None else []
                limit = 2 if str(ins.opcode) == "EventSemaphore" else 1
                if len(waits) > limit:
                    extra, keep = waits[:-limit], waits[-limit:]
                    for w in extra:
                        uid += 1
                        out.append(
                            mybir.InstNoOp(
                                name=f"waitsplit-{uid}",
                                engine=ins.engine,
                                sync_info=mybir.SyncInfo(on_wait=[w], on_update=[]),
                                bass_nofuse=True,
                            )
                        )
                    si.on_wait = keep
                    changed = True
                out.append(ins)
            if changed:
                insts.clear()
                insts.extend(out)


def _patch_tail_barrier(tile_mod):
    """Tile's kernel tail is drain -> barrier -> sem-reset -> barrier.
    The second all-engine barrier only orders the sem-reset against program
    end, which the per-engine stream end already guarantees; drop it."""
    if getattr(tile_mod.TileContext, "_moe_tail_patched", False):
        return
    from concourse.vector_clock import ScopedClock

    def _drain_and_barrier(self, tick_clock, wait_clock):
        drain_inst = self.nc.sync.drain()
        wait_clock.add_sem_waits(
            drain_inst.ins, ScopedClock({None: tick_clock.global_clock})
        )
        self.nc.all_engine_barrier()
        popped = self.nc._tile_sem_poison_stack.pop()
        assert popped is self._sem_poison
        self.nc.clear_and_free_semaphores(list(self.sems.allocated().values()))

    tile_mod.TileContext._drain_and_barrier = _drain_and_barrier
    tile_mod.TileContext._moe_tail_patched = True


def _patch_walrus_policy():
    """Compile with walrus --policy=2 (heuristics post-scheduler): measured
    faster than the default --policy=0 on this kernel family."""
    import concourse.bass_utils as bu

    if getattr(bu, "_moe_policy_patched", False):
        return
    orig = bu.run_command

    def _rc(argv, **kw):
        if argv and "walrus_driver" in str(argv[0]):
            argv = ["--policy=2" if a == "--policy=0" else a for a in argv]
        return orig(argv, **kw)

    bu.run_command = _rc
    bu._moe_policy_patched = True


def _emit(nc, tile, mm_dt, f32):
    fwa = nc.dram_tensor("fwa", [P, KO_O, KO_D, P], mm_dt, kind="ExternalInput")
    fwb = nc.dram_tensor("fwb", [P, KO_O, KO_D, P], mm_dt, kind="ExternalInput")
    xa = nc.dram_tensor("xa", [P, KO_D, CAP_A], mm_dt, kind="ExternalInput")
    xb = nc.dram_tensor("xb", [P, KO_D, CAP_B], mm_dt, kind="ExternalInput")
    outa = nc.dram_tensor("outa", [P, KO_O, CAP_A], mm_dt, kind="ExternalOutput")
    outb = nc.dram_tensor("outb", [P, KO_O, CAP_B], mm_dt, kind="ExternalOutput")

    with tile.TileContext(nc) as tc:
        with ExitStack() as ctx:
            fw_pool = ctx.enter_context(tc.tile_pool(name="fw", bufs=1))
            x_pool = ctx.enter_context(tc.tile_pool(name="x", bufs=1))
            o_pool = ctx.enter_context(tc.tile_pool(name="o", bufs=1))
            jk_pool = ctx.enter_context(tc.tile_pool(name="jk", bufs=1))
            psa_pool = ctx.enter_context(
                tc.tile_pool(name="psa", bufs=5, space="PSUM")
            )
            ps_pool = ctx.enter_context(
                tc.tile_pool(name="ps", bufs=3, space="PSUM")
            )

            fwa_sb = fw_pool.tile([P, KO_O, KO_D, P], mm_dt, name="fwa")
            fwb_sb = fw_pool.tile([P, KO_O, KO_D, P], mm_dt, name="fwb")
            xa_sb = x_pool.tile([P, KO_D, CAP_A], mm_dt, name="xa")
            xb_sb = x_pool.tile([P, KO_D, CAP_B], mm_dt, name="xb")
            outa_sb = o_pool.tile([P, KO_O, CAP_A], mm_dt, name="outa")
            outb_sb = o_pool.tile([P, KO_O, CAP_B], mm_dt, name="outb")
            jst = jk_pool.tile([P, P], mm_dt, name="jst")
            jmv = jk_pool.tile([P, 256], mm_dt, name="jmv")
            jps = psa_pool.tile([P, 512], f32, name="psa")

            # ---- junk warm-up: ramp the PE p-state while DMA streams in
            nc.vector.memset(jst[:], 0)
            nc.vector.memset(jmv[:], 0)
            for _ in range(N_JUNK):
                nc.tensor.matmul(jps[:, :256], jst[:], jmv[:], start=True,
                                 stop=True)
            for _ in range(N_JUNK64):
                nc.tensor.matmul(jps[:, :64], jst[:], jmv[:, :64], start=True,
                                 stop=True)

            # ---- input DMAs: ONE queue (sync), strict demand order. The
            # HWDGE ring paces issues to transfer completions and queues
            # share the ~360GB/s HBM port, so a single in-order stream beats
            # splitting (a second queue halves the critical stream's BW).
            # Output DMAs go on the scalar queue so they never block inputs.
            c01 = GA[0]
            c12 = GA[0] + GA[1]
            c23 = GA[0] + GA[1] + GA[2]
            # first visit gated on just 0.375MB: half of fwa ot0 + 128 cols;
            # stream strictly in first-need order
            nc.sync.dma_start(fwa_sb[:, 0, 0:4], fwa[:, 0, 0:4])
            nc.sync.dma_start(xa_sb[:, :, :c01], xa[:, :, :c01])
            nc.sync.dma_start(fwa_sb[:, 0, 4:8], fwa[:, 0, 4:8])
            nc.sync.dma_start(fwa_sb[:, 1], fwa[:, 1])
            nc.sync.dma_start(xa_sb[:, :, c01:c12], xa[:, :, c01:c12])
            nc.sync.dma_start(fwa_sb[:, 2], fwa[:, 2])
            nc.sync.dma_start(fwa_sb[:, 3], fwa[:, 3])
            nc.sync.dma_start(xa_sb[:, :, c12:c23], xa[:, :, c12:c23])
            for h in range(4, KO_O):
                nc.sync.dma_start(fwa_sb[:, h], fwa[:, h])
            nc.sync.dma_start(xa_sb[:, :, c23:], xa[:, :, c23:])
            nc.sync.dma_start(xb_sb[:], xb[:])
            nc.sync.dma_start(fwb_sb[:, 0:4], fwb[:, 0:4])
            nc.sync.dma_start(fwb_sb[:, 4:8], fwb[:, 4:8])

            groups = [  # (x tile, col0, width, fw tile, out tile)
                (xa_sb, 0, GA[0], fwa_sb, outa_sb),
                (xa_sb, c01, GA[1], fwa_sb, outa_sb),
                (xa_sb, c12, GA[2], fwa_sb, outa_sb),
                (xa_sb, c23, GA[3], fwa_sb, outa_sb),
                (xb_sb, 0, CAP_B, fwb_sb, outb_sb),
            ]
            psA = {}             # ot -> shared PSUM bank for groups a,b,c
            ddone = [False] * KO_O
            for ot, g in VISITS:
                x_sb, c0, gw, fw_sb, out_sb = groups[g]
                if g < 3:
                    if g == 0:
                        psA[ot] = psa_pool.tile([P, 512], f32, name="psa")
                    ps = psA[ot][:, c0 : c0 + gw]
                else:
                    ps = ps_pool.tile([P, 512], f32, name="ps")[:, :gw]
                for k in range(KO_D):
                    nc.tensor.matmul(
                        ps,
                        fw_sb[:, ot, k],
                        x_sb[:, k, c0 : c0 + gw] if g < 4 else x_sb[:, k],
                        start=(k == 0),
                        stop=(k == KO_D - 1),
                    )
                # copies on DVE only: gpsimd/Pool can't read PSUM on trn2, and
                # scalar ACTIVATE would pull in a 1.3us ACT_TABLE_LOAD that
                # delays the scalar queue's DMA issues
                if g == 2:  # a,b,c of this ot all accumulated -> one copy
                    nc.vector.tensor_copy(outa_sb[:, ot, :c23], psA.pop(ot)[:])
                elif g == 3:
                    nc.vector.tensor_copy(outa_sb[:, ot, c23:], ps)
                    ddone[ot] = True
                    pair = ot & ~1
                    if ddone[pair] and ddone[pair + 1]:
                        nc.scalar.dma_start(
                            outa[:, pair : pair + 2], outa_sb[:, pair : pair + 2]
                        )
                elif g == 4:
                    nc.vector.tensor_copy(outb_sb[:, ot, :], ps)
                    # outb rides the idle sync queue, in two halves so the
                    # final transfer after the last visit is small
                    if ot == 3:
                        nc.sync.dma_start(outb[:, 0:4], outb_sb[:, 0:4])
                    elif ot == 7:
                        nc.sync.dma_start(outb[:, 4:8], outb_sb[:, 4:8])
    return nc


def _build_nc():
    import concourse.bass as bass
    import concourse.tile as tile
    from concourse import mybir

    _patch_tail_barrier(tile)
    _patch_walrus_policy()
    nc = bass.Bass()
    _emit(nc, tile, mybir.dt.bfloat16, mybir.dt.float32)
    _legalize_waits(nc)
    return nc


_NC_CACHE = {}


def kernel(x, gate_w, gate_b, expert_w, expert_b, out_w, out_b):
    import os

    plats = os.environ.get("JAX_PLATFORMS")
    if plats and "axon" not in plats:
        os.environ["JAX_PLATFORMS"] = plats + ",axon"

    import ml_dtypes
    from concourse.bass_utils import run_bass_kernel_spmd

    bf = ml_dtypes.bfloat16
    x = np.asarray(x, dtype=np.float32)
    gate_w = np.asarray(gate_w, dtype=np.float32)
    gate_b = np.asarray(gate_b, dtype=np.float32)
    expert_w = np.asarray(expert_w, dtype=np.float32)
    expert_b = np.asarray(expert_b, dtype=np.float32)
    out_w = np.asarray(out_w, dtype=np.float32)
    out_b = np.asarray(out_b, dtype=np.float32)

    xt = x.reshape(T, D)
    # Gate on host in fp64: argmax matches the fp32 reference exactly
    # (min top-2 logit gap is ~1e-5, fp64 error ~1e-12).
    logits = xt.astype(np.float64) @ gate_w.astype(np.float64) + gate_b.astype(
        np.float64
    )
    idx = np.argmax(logits, axis=1)

    # Weight-only fusion (input-independent): FW[e] = W[e] @ OW, and the
    # bias path bias2[e] = b[e] @ OW + ob, both applied per routed token.
    FW = (expert_w.reshape(E * D, H) @ out_w).reshape(E, D, O)
    bias2 = (
        expert_b.astype(np.float64) @ out_w.astype(np.float64)
        + out_b.astype(np.float64)
    ).astype(np.float32)

    # fw packed [P, KO_O, KO_D, P]: fw[p, ot, kd, j] = FW[kd*128+p, ot*128+j]
    fw_pk = [
        np.ascontiguousarray(
            FW[e].astype(bf).reshape(KO_D, P, KO_O, P).transpose(1, 2, 0, 3)
        )
        for e in range(E)
    ]

    # ---- token -> core assignment: home expert e on core e (CAP_A tokens),
    # spillover chopped into <=CAP_B chunks placed in other cores' B slots.
    tok_of_expert = [np.nonzero(idx == e)[0] for e in range(E)]
    home = [t[:CAP_A] for t in tok_of_expert]
    ovf = [(e, tok_of_expert[e][CAP_A:]) for e in range(E)]
    chunks = []
    for e, t in sorted(ovf, key=lambda p: -len(p[1])):
        for i in range(0, len(t), CAP_B):
            chunks.append((e, t[i : i + CAP_B]))
    chunks = [c for c in chunks if len(c[1])]
    host_fb = chunks[E:]          # shouldn't happen for this routing
    chunks = chunks[:E]
    while len(chunks) < E:
        chunks.append((len(chunks), np.empty(0, dtype=np.int64)))

    def pack_x(toks, cap):
        cols = np.zeros((cap, D), dtype=bf)
        if len(toks):
            cols[: len(toks)] = xt[toks].astype(bf)
        # [cap, D] -> [P, KO_D, cap]
        return np.ascontiguousarray(
            cols.T.reshape(KO_D, P, cap).transpose(1, 0, 2)
        )

    in_maps = []
    for e in range(E):
        be, bt = chunks[e]
        in_maps.append(
            {
                "fwa": fw_pk[e],
                "fwb": fw_pk[be],
                "xa": pack_x(home[e], CAP_A),
                "xb": pack_x(bt, CAP_B),
            }
        )

    if "nc" not in _NC_CACHE:
        _NC_CACHE["nc"] = _build_nc()
    nc = _NC_CACHE["nc"]

    res = run_bass_kernel_spmd(nc, in_maps, list(range(E)))

    out = np.empty((T, O), dtype=np.float32)
    for e in range(E):
        # dev out [P, KO_O, cap] -> [cap, O] with col = ot*128+p
        oa = (
            np.asarray(res.results[e]["outa"])
            .transpose(2, 1, 0)
            .reshape(CAP_A, O)
            .astype(np.float32)
        )
        out[home[e]] = oa[: len(home[e])] + bias2[e]
        be, bt = chunks[e]
        if len(bt):
            ob_ = (
                np.asarray(res.results[e]["outb"])
                .transpose(2, 1, 0)
                .reshape(CAP_B, O)
                .astype(np.float32)
            )
            out[bt] = ob_[: len(bt)] + bias2[be]
    for e, toks in host_fb:
        out[toks] = xt[toks] @ FW[e] + bias2[e]
    return out.reshape(B, S, O)
